# revision 1
# baseline (speedup 1.0000x reference)
"""Causal self-attention (B=4, T=2048, D=1024, H=16) on 8 NeuronCores.

Sharding: core c handles batch b=c//2 and head-group hg=c%2 (8 of 16 heads).
Per core: column-parallel Wq/Wk/Wv (512 cols), row-parallel Wo (512 rows).
Host sums the two partial outputs per batch and adds bo. No collectives.

On-chip layout (all transposed; no on-chip transposes needed):
  xT [D=1024, T=2048] (host pre-transposed), qT/kT [512 dout, T],
  V natural [T, 8 heads x (64 dv + 1 ones col)].
  Scores computed as S^T [t_k, t_q] = kT.T @ qT; exp (no max subtraction --
  scores are O(4), fp32 exp safe); PV matmul out^T[dv, t_q] = V_aug.T @ expS^T
  with the ones column yielding sumexp for free; divide via batched
  reciprocal + K=1 broadcast matmul; final projection consumes out^T
  directly as lhsT.

Dtypes: MODE="safe": qk projections + scores in float32r (fp32 rounded to
11-bit mantissa; 2 cyc/row on PE), V/PV/final chain in bf16 (1 cyc/row).
MODE="fast": everything bf16.
Diagonal k-tiles narrow their matmul/exp/mask N to the causally valid
column range (columns < o are fully masked in S^T tile at offset o).
"""

import os
from contextlib import ExitStack

import ml_dtypes
import numpy as np

import concourse.bacc as bacc
import concourse.mybir as mybir
import concourse.tile as tile
from concourse.bass_utils import run_bass_kernel_spmd

B, T, D, H, DK = 4, 2048, 1024, 16, 64
HL = 8  # heads per core
CD = HL * DK  # 512 local channels
NP = 128  # partitions
QB = 512  # query block / matmul moving dim
NDC = D // NP  # 8 din chunks
NTT = T // NP  # 16 t-tiles
NTB = T // QB  # 4 t-blocks
NPAIR = HL // 2  # 4 head pairs
F32 = mybir.dt.float32
F32R = mybir.dt.float32r
BF16 = mybir.dt.bfloat16
Exp = mybir.ActivationFunctionType.Exp
Identity = mybir.ActivationFunctionType.Identity

MODE = os.environ.get("KERNEL_MODE", "safe")

_CACHE: dict = {}


def _build_nc():
    DTQ = BF16 if MODE == "fast" else F32R  # xt / wq / wk / qt / kt / scores
    nc = bacc.Bacc("TRN2", target_bir_lowering=False, debug=False)
    xt = nc.dram_tensor("xt", [D, T], DTQ, kind="ExternalInput")
    wq = nc.dram_tensor("wq", [D, CD], DTQ, kind="ExternalInput")
    wk = nc.dram_tensor("wk", [D, CD], DTQ, kind="ExternalInput")
    wv = nc.dram_tensor("wv", [D, CD], BF16, kind="ExternalInput")
    wo = nc.dram_tensor("wo", [CD, D], BF16, kind="ExternalInput")
    bqc = nc.dram_tensor("bqc", [NP, NPAIR], F32, kind="ExternalInput")
    bkc = nc.dram_tensor("bkc", [NP, NPAIR], F32, kind="ExternalInput")
    bvr = nc.dram_tensor("bvr", [1, CD], BF16, kind="ExternalInput")
    msk = nc.dram_tensor("msk", [4, NP, QB], F32, kind="ExternalInput")
    onesd = nc.dram_tensor("onesd", [NP, QB], BF16, kind="ExternalInput")
    y = nc.dram_tensor("y", [T, D], F32, kind="ExternalOutput")

    with tile.TileContext(nc) as tc, ExitStack() as ctx:
        _body(nc, tc, ctx, DTQ, xt, wq, wk, wv, wo, bqc, bkc, bvr, msk, onesd, y)
    nc.compile()
    return nc


def _body(nc, tc, ctx, DTQ, xt, wq, wk, wv, wo, bqc, bkc, bvr, msk, onesd, y):
    const = ctx.enter_context(tc.tile_pool(name="const", bufs=1))
    vpool = ctx.enter_context(tc.tile_pool(name="v", bufs=1))
    oatp = ctx.enter_context(tc.tile_pool(name="oat", bufs=1))
    xtp = ctx.enter_context(tc.tile_pool(name="xt", bufs=9))
    # PSUM: proj(2, shared w/ bcast) + score(2 tags x 2) + pv(2 tags x 1) = 8
    projps = ctx.enter_context(tc.tile_pool(name="projps", bufs=2, space="PSUM"))
    scoreps = ctx.enter_context(tc.tile_pool(name="scoreps", bufs=2, space="PSUM"))
    pvps = ctx.enter_context(tc.tile_pool(name="pvps", bufs=1, space="PSUM"))

    # constants
    ones_t = const.tile([1, QB], BF16)
    nc.sync.dma_start(ones_t[:], onesd[0:1, :])
    bq_sb = const.tile([NP, NPAIR], F32, tag="bq")
    nc.sync.dma_start(bq_sb[:], bqc[:])
    bk_sb = const.tile([NP, NPAIR], F32, tag="bk")
    nc.sync.dma_start(bk_sb[:], bkc[:])
    bv_sb = const.tile([1, CD], BF16, tag="bv")
    nc.sync.dma_start(bv_sb[:], bvr[:])
    msk_sb = const.tile([NP, 4, QB], F32, tag="msk")
    for o in range(4):
        nc.sync.dma_start(msk_sb[:, o, :], msk[o, :, :])
    ones_f = const.tile([1, DK], F32, tag="onesf")
    nc.vector.memset(ones_f[:], 1.0)
    # warm up the exp table set early (one-time ~2.7us load overlaps V phase)
    warm = const.tile([1, 2], F32, tag="warm")
    nc.vector.memset(warm[:], 0.0)
    nc.scalar.activation(warm[:], warm[:], Exp)

    # ---- V phase: V[t, dv] for all 8 heads (bf16), with ones column ----
    v_sb = [
        vpool.tile([NP, HL, DK + 1], BF16, tag=f"v{tt}", name=f"v{tt}")
        for tt in range(NTT)
    ]
    wvp_cm = tc.tile_pool(name="wvp", bufs=1)
    wvp = wvp_cm.__enter__()
    wv_sb = wvp.tile([NP, NDC, CD], BF16, tag="wv")
    for d in range(NDC):
        nc.sync.dma_start(wv_sb[:, d, :], wv[d * NP : (d + 1) * NP, :])
    for tb in range(NTB):
        xvts = []
        for d in range(NDC):
            xvtile = wvp.tile([NP, QB], BF16, tag="xv", bufs=9, name="xv")
            src = xt[d * NP : (d + 1) * NP, tb * QB : (tb + 1) * QB]
            if DTQ == BF16:
                nc.sync.dma_start(xvtile[:], src)
            else:
                nc.gpsimd.dma_start(xvtile[:], src.bitcast(F32))  # cast f32->bf16
            xvts.append(xvtile)
        for i in range(QB // NP):
            tt = tb * (QB // NP) + i
            ps = projps.tile([NP, CD], F32, tag="proj")
            for d in range(NDC):
                nc.tensor.matmul(
                    ps[:],
                    xvts[d][:, i * NP : (i + 1) * NP],
                    wv_sb[:, d, :],
                    start=(d == 0),
                    stop=False,
                )
            # + ones_col x bv  (bias along free dim via K=1 rank-1 update)
            nc.tensor.matmul(
                ps[:], ones_t[0:1, 0:NP], bv_sb[:], start=False, stop=True
            )
            vt = v_sb[tt]
            nc.sync.dma_start(vt[:, :, DK : DK + 1], onesd[:, 0:HL])
            nc.vector.tensor_copy(vt[:, :, 0:DK], ps.rearrange("p (h k) -> p h k", h=HL))

    wvp_cm.__exit__(None, None, None)

    # ---- per head-pair: qT/kT projection then attention ----
    wqkp = ctx.enter_context(tc.tile_pool(name="wqk", bufs=2))
    qtp = ctx.enter_context(tc.tile_pool(name="qt", bufs=2))
    ktp = ctx.enter_context(tc.tile_pool(name="kt", bufs=2))
    expp = ctx.enter_context(tc.tile_pool(name="exp", bufs=2))
    smallp = ctx.enter_context(tc.tile_pool(name="small", bufs=2))
    oat = [oatp.tile([NP, T], BF16, tag=f"oat{c}", name=f"oat{c}") for c in range(NPAIR)]

    for c in range(NPAIR):
        wqc = wqkp.tile([NP, NDC, NP], DTQ, tag="wqc")
        wkc = wqkp.tile([NP, NDC, NP], DTQ, tag="wkc")
        for d in range(NDC):
            nc.sync.dma_start(
                wqc[:, d, :], wq[d * NP : (d + 1) * NP, c * NP : (c + 1) * NP]
            )
            nc.sync.dma_start(
                wkc[:, d, :], wk[d * NP : (d + 1) * NP, c * NP : (c + 1) * NP]
            )
        qt = qtp.tile([NP, T], DTQ)
        kt_t = ktp.tile([NP, T], DTQ)
        for tb in range(NTB):
            xts = []
            for d in range(NDC):
                xtile = xtp.tile([NP, QB], DTQ)
                nc.sync.dma_start(
                    xtile[:], xt[d * NP : (d + 1) * NP, tb * QB : (tb + 1) * QB]
                )
                xts.append(xtile)
            psq = projps.tile([NP, QB], F32, tag="proj")
            for d in range(NDC):
                nc.tensor.matmul(
                    psq[:], wqc[:, d, :], xts[d][:],
                    start=(d == 0), stop=(d == NDC - 1),
                )
            nc.scalar.activation(
                qt[:, tb * QB : (tb + 1) * QB], psq[:], Identity,
                bias=bq_sb[:, c : c + 1],
            )
            psk = projps.tile([NP, QB], F32, tag="proj")
            for d in range(NDC):
                nc.tensor.matmul(
                    psk[:], wkc[:, d, :], xts[d][:],
                    start=(d == 0), stop=(d == NDC - 1),
                )
            nc.scalar.activation(
                kt_t[:, tb * QB : (tb + 1) * QB], psk[:], Identity,
                bias=bk_sb[:, c : c + 1],
            )

        # attention for this pair
        for qb in range(NTB):
            nkt = 4 * qb + 4  # k-tiles 0..4qb+3 (last 4 are diagonal)
            pv = [
                pvps.tile([DK + 1, QB], F32, tag=f"pv{h}", name=f"pv{h}")
                for h in range(2)
            ]
            for kti in range(nkt):
                di = kti - 4 * qb  # >=0 on diagonal tiles
                o = max(di, 0) * NP  # first causally valid column
                sps = [
                    scoreps.tile([NP, QB], F32, tag=f"s{h}", name=f"s{h}")
                    for h in range(2)
                ]
                for h in range(2):
                    nc.tensor.matmul(
                        sps[h][:, o:QB],
                        kt_t[64 * h : 64 * h + 64, kti * NP : (kti + 1) * NP],
                        qt[64 * h : 64 * h + 64, qb * QB + o : (qb + 1) * QB],
                        start=True, stop=True,
                        tile_position=(64 * h, 0),
                    )
                for h in range(2):
                    et = expp.tile([NP, QB], BF16, tag=f"e{h}", name=f"e{h}")
                    if di >= 0:
                        tmp = expp.tile([NP, QB], F32, tag="tmp")
                        nc.vector.tensor_add(
                            tmp[:, o:QB], sps[h][:, o:QB], msk_sb[:, di, o:QB]
                        )
                        nc.scalar.activation(
                            et[:, o:QB], tmp[:, o:QB], Exp, scale=0.125
                        )
                    else:
                        nc.scalar.activation(
                            et[:, o:QB], sps[h][:, o:QB], Exp, scale=0.125
                        )
                    hh = 2 * c + h
                    nc.tensor.matmul(
                        pv[h][:, o:QB],
                        v_sb[kti][:, hh, :],
                        et[:, o:QB],
                        start=(kti == 0), stop=(kti == nkt - 1),
                    )
            for h in range(2):
                recip = smallp.tile([1, QB], F32, tag="recip")
                nc.vector.reciprocal(recip[:], pv[h][DK : DK + 1, :])
                bc = projps.tile([NP, QB], F32, tag="proj")
                nc.tensor.matmul(
                    bc[0:DK, :], ones_f[0:1, 0:DK], recip[:],
                    start=True, stop=True,
                )
                bcs = smallp.tile([DK, QB], F32, tag="bcs")
                nc.vector.tensor_copy(bcs[:], bc[0:DK, :])
                nc.vector.tensor_mul(
                    oat[c][64 * h : 64 * h + 64, qb * QB : (qb + 1) * QB],
                    pv[h][0:DK, :],
                    bcs[:],
                )

    # ---- final projection: y[t, dout] = outAllT.T @ Wo ----
    wop = ctx.enter_context(tc.tile_pool(name="wop", bufs=1))
    wo_sb = wop.tile([NP, NDC // 2, D], BF16, tag="wo")
    for c in range(NPAIR):
        nc.sync.dma_start(wo_sb[:, c, :], wo[c * NP : (c + 1) * NP, :])
    for tt in range(NTT):
        for dh in range(2):
            ps = projps.tile([NP, QB], F32, tag="proj")
            for c in range(NPAIR):
                nc.tensor.matmul(
                    ps[:],
                    oat[c][:, tt * NP : (tt + 1) * NP],
                    wo_sb[:, c, dh * QB : (dh + 1) * QB],
                    start=(c == 0), stop=(c == NPAIR - 1),
                )
            ystage = smallp.tile([NP, QB], F32, tag="ystage", bufs=2)
            nc.vector.tensor_copy(ystage[:], ps[:])
            nc.sync.dma_start(
                y[tt * NP : (tt + 1) * NP, dh * QB : (dh + 1) * QB], ystage[:]
            )


def _install_ntff_hook_shim():
    """The agent image's antenv lacks axon_hooks, so trace=True under axon
    degrades. Provide the missing module and register the ctypes NTFF hook
    from trn_agent_boot. Best-effort: failures just mean no trace."""
    try:
        import sys
        import types

        if "antenv.axon_hooks" not in sys.modules:
            mod = types.ModuleType("antenv.axon_hooks")
            mod._hook = None
            mod.set_axon_ntff_profile_hook = lambda h: setattr(mod, "_hook", h)
            mod.get_axon_ntff_profile_hook = lambda: mod._hook
            sys.modules["antenv.axon_hooks"] = mod
            import antenv

            antenv.axon_hooks = mod
        from antenv.axon_hooks import (
            get_axon_ntff_profile_hook,
            set_axon_ntff_profile_hook,
        )

        if get_axon_ntff_profile_hook() is None:
            from trn_agent_boot.trn_boot import _ntff_profile_via_ctypes

            hook = _ntff_profile_via_ctypes("/opt/axon/libaxon_pjrt.so")
            if hook is not None:
                set_axon_ntff_profile_hook(hook)
    except Exception as e:  # noqa: BLE001
        print(f"ntff hook shim failed ({e}); running without trace")


def _round_f32r(a: np.ndarray) -> np.ndarray:
    """Round fp32 to fp32r (11-bit mantissa, low 12 bits zero), RNE."""
    u = np.ascontiguousarray(a, dtype=np.float32).view(np.uint32)
    u = (u + np.uint32(0x7FF) + ((u >> np.uint32(12)) & np.uint32(1))) & np.uint32(
        0xFFFFF000
    )
    return u.view(np.float32)


def _qdt(a: np.ndarray) -> np.ndarray:
    if MODE == "fast":
        return np.ascontiguousarray(a, dtype=np.float32).astype(ml_dtypes.bfloat16)
    return _round_f32r(a)


def _bf(a: np.ndarray) -> np.ndarray:
    return np.ascontiguousarray(a, dtype=np.float32).astype(ml_dtypes.bfloat16)


def _make_masks() -> np.ndarray:
    m = np.zeros((4, NP, QB), dtype=np.float32)
    kk = np.arange(NP)[:, None]
    qq = np.arange(QB)[None, :]
    for o in range(4):
        m[o] = np.where(qq >= kk + o * NP, 0.0, -1e30)
    return m


def kernel(x, Wq, bq, Wk, bk, Wv, bv, Wo, bo):
    x = np.ascontiguousarray(np.asarray(x, dtype=np.float32))
    Wq, bq = np.asarray(Wq, np.float32), np.asarray(bq, np.float32)
    Wk, bk = np.asarray(Wk, np.float32), np.asarray(bk, np.float32)
    Wv, bv = np.asarray(Wv, np.float32), np.asarray(bv, np.float32)
    Wo, bo = np.asarray(Wo, np.float32), np.asarray(bo, np.float32)

    if "nc" not in _CACHE:
        _CACHE["nc"] = _build_nc()
    nc = _CACHE["nc"]

    masks = _make_masks()
    ones_bf = np.ones((NP, QB), dtype=ml_dtypes.bfloat16)
    in_maps = []
    for core in range(8):
        b, hg = core // 2, core % 2
        cs = slice(hg * CD, (hg + 1) * CD)
        in_maps.append(
            {
                "xt": _qdt(x[b].T),
                "wq": _qdt(Wq[:, cs]),
                "wk": _qdt(Wk[:, cs]),
                "wv": _bf(Wv[:, cs]),
                "wo": _bf(Wo[cs, :]),
                "bqc": np.ascontiguousarray(bq[cs].reshape(NPAIR, NP).T),
                "bkc": np.ascontiguousarray(bk[cs].reshape(NPAIR, NP).T),
                "bvr": _bf(bv[cs].reshape(1, CD)),
                "msk": masks,
                "onesd": ones_bf,
            }
        )

    trace = bool(os.environ.get("KERNEL_TRACE"))
    if trace:
        _install_ntff_hook_shim()
    res = run_bass_kernel_spmd(
        nc, in_maps, core_ids=list(range(8)), trace=trace
    )
    _CACHE["last_results"] = res

    out = np.empty((B, T, D), dtype=np.float32)
    for b in range(B):
        out[b] = res.results[2 * b]["y"] + res.results[2 * b + 1]["y"] + bo
    return out



# revision 26
# speedup vs baseline: 1.5711x; 1.5711x over previous
"""Causal self-attention (B=4, T=2048, D=1024, H=16) on 8 NeuronCores.

Sharding: core c handles batch b=c//2 and head-group hg=c%2 (8 of 16 heads).
Per core: column-parallel Wq/Wk/Wv (512 cols), row-parallel Wo (512 rows).
Host sums the two partial outputs per batch and adds bo_eff. No collectives.

Structure (v2 - optimized):
  - x^T resident in SBUF as fp32r (64KB/partition), loaded once with big DMAs.
  - All biases eliminated from the kernel:
      * bk and all per-query additive score terms drop (softmax invariance).
      * bq.k_raw[tk] enters as a per-partition exp bias, computed by an 8-wide
        side matmul chain (x @ (0.125*Wk_h@bq_h)) sharing the V-phase weights.
      * bv commutes through attention; host folds bv@Wo into bo.
  - Scores S^T[tk,tq] = kt.T @ qt per 128-k-tile, two heads packed in the PE
    array via row groups (K=64 each). fp32r operands; diagonal tiles keep
    moving-dim >= 256 (fp32r narrow-N penalty).
  - exp on ACT with scale=0.125 and bqk bias; causal triangle zeroed post-exp
    with a shared 128x128 0/1 bf16 mask on DVE.
  - PV accumulates unnormalized out^T plus a sumexp row via a ones column in
    V (M=65). Division deferred: one reciprocal_approx_fast per pair on a
    [8,512] tile, K=8 selector matmul broadcasts 1/d to both heads' partitions.
  - Next pair's Q/K projection matmuls are interleaved as PE filler inside the
    attention loop so the PE stays dense (HAM-warm) while ACT runs exp.
  - Final projection y = oat.T @ Wo at the end; y output in bf16.
"""

import os
from contextlib import ExitStack

import ml_dtypes
import numpy as np

import concourse.bacc as bacc
import concourse.mybir as mybir
import concourse.tile as tile
from concourse.bass_utils import run_bass_kernel_spmd

B, T, D, H, DK = 4, 2048, 1024, 16, 64
HL = 8  # heads per core
CD = HL * DK  # 512 local channels
NP = 128  # partitions
QB = 512  # query block
NDC = D // NP  # 8 din chunks
NTT = T // NP  # 16 t-tiles
NTB = T // QB  # 4 t-blocks
NPAIR = HL // 2  # 4 head pairs
VW = DK + 2  # v row stride (64 data + 1 ones + 1 pad for 4B alignment)
F32 = mybir.dt.float32
F32R = mybir.dt.float32r
BF16 = mybir.dt.bfloat16
Exp = mybir.ActivationFunctionType.Exp

_CACHE: dict = {}


def _build_nc():
    nc = bacc.Bacc("TRN2", target_bir_lowering=False, debug=False)
    xt = nc.dram_tensor("xt", [D, T], BF16, kind="ExternalInput")
    wq = nc.dram_tensor("wq", [D, CD], BF16, kind="ExternalInput")
    wk = nc.dram_tensor("wk", [D, CD], BF16, kind="ExternalInput")
    wv = nc.dram_tensor("wv", [D, CD], BF16, kind="ExternalInput")
    wkb = nc.dram_tensor("wkb", [D, HL], BF16, kind="ExternalInput")
    wo = nc.dram_tensor("wo", [CD, D], BF16, kind="ExternalInput")
    tri = nc.dram_tensor("tri", [NP, NP], BF16, kind="ExternalInput")
    y = nc.dram_tensor("y", [T, D], BF16, kind="ExternalOutput")

    with tile.TileContext(nc) as tc, ExitStack() as ctx:
        _body(nc, tc, ctx, xt, wq, wk, wv, wkb, wo, tri, y)
    nc.compile()
    return nc


def _body(nc, tc, ctx, xt, wq, wk, wv, wkb, wo, tri, y):
    const = ctx.enter_context(tc.tile_pool(name="const", bufs=1))
    xtp = ctx.enter_context(tc.tile_pool(name="xt", bufs=1))
    vpool = ctx.enter_context(tc.tile_pool(name="v", bufs=1))
    bqkp = ctx.enter_context(tc.tile_pool(name="bqk", bufs=1))
    oatp = ctx.enter_context(tc.tile_pool(name="oat", bufs=1))
    wqkp = ctx.enter_context(tc.tile_pool(name="wqk", bufs=2))
    qkp = ctx.enter_context(tc.tile_pool(name="qk", bufs=2))
    etp = ctx.enter_context(tc.tile_pool(name="et", bufs=2))
    pvsp = ctx.enter_context(tc.tile_pool(name="pvs", bufs=2))
    dnp = ctx.enter_context(tc.tile_pool(name="dn", bufs=2))
    smallp = ctx.enter_context(tc.tile_pool(name="small", bufs=2))
    # PSUM pools are phase-scoped to stay within 8 banks:
    #   V phase:   proj(2) + bqkps(2) = 4
    #   attention: proj(2) + scores(4) + pv(2) = 8
    #   tail:      proj(2) + y(4) = 6
    projps = ctx.enter_context(tc.tile_pool(name="projps", bufs=2, space="PSUM"))

    # ---- constants / one-time ----
    tri_sb = const.tile([NP, NP], BF16, tag="tri")
    nc.sync.dma_start(tri_sb[:], tri[:])
    # selector rows for the 1/d broadcast: head h covers partitions 64h..64h+63
    sel64 = []
    for h in range(2):
        st = const.tile([1, NP], BF16, tag=f"sel{h}", name=f"sel{h}")
        nc.vector.memset(st[:], 0.0)
        nc.vector.memset(st[0:1, 64 * h : 64 * h + 64], 1.0)
        sel64.append(st)
    # warm the exp table set early
    warm = const.tile([1, 2], F32, tag="warm")
    nc.vector.memset(warm[:], 0.0)
    nc.scalar.activation(warm[:], warm[:], Exp)

    # ---- resident x^T (fp32r), 16 half-chunk DMAs ----
    xt_sb = xtp.tile([NP, NDC, T], BF16, tag="xt")
    for d in range(NDC):
        for hlf in range(2):
            sl = slice(hlf * (T // 2), (hlf + 1) * (T // 2))
            nc.sync.dma_start(xt_sb[:, d, sl], xt[d * NP : (d + 1) * NP, sl])

    # ---- V phase: V[t, h, dv] bf16 + ones col; bqk side chain ----
    v_sb = [
        vpool.tile([NP, HL, VW], BF16, tag=f"v{tt}", name=f"v{tt}")
        for tt in range(NTT)
    ]
    bqk_sb = bqkp.tile([NP, NTT, HL], F32, tag="bqk")
    wvp_cm = tc.tile_pool(name="wvp", bufs=1)
    wvp = wvp_cm.__enter__()
    bqkps_cm = tc.tile_pool(name="bqkps", bufs=2, space="PSUM")
    bqkps = bqkps_cm.__enter__()
    wv_sb = wvp.tile([NP, NDC, CD], BF16, tag="wv")
    wkb_sb = wvp.tile([NP, NDC, HL], BF16, tag="wkb")
    for d in range(NDC):
        nc.sync.dma_start(wv_sb[:, d, :], wv[d * NP : (d + 1) * NP, :])
        nc.sync.dma_start(wkb_sb[:, d, :], wkb[d * NP : (d + 1) * NP, :])
    # pair-0 projection weights land during V phase
    wq_t = [None] * NPAIR
    wk_t = [None] * NPAIR

    def emit_wqk_dma(c):
        wq_t[c] = wqkp.tile([NP, NDC, NP], BF16, tag="wqc", name="wqc")
        wk_t[c] = wqkp.tile([NP, NDC, NP], BF16, tag="wkc", name="wkc")
        for d in range(NDC):
            nc.gpsimd.dma_start(
                wq_t[c][:, d, :], wq[d * NP : (d + 1) * NP, c * NP : (c + 1) * NP]
            )
            nc.gpsimd.dma_start(
                wk_t[c][:, d, :], wk[d * NP : (d + 1) * NP, c * NP : (c + 1) * NP]
            )

    emit_wqk_dma(0)

    for tt in range(NTT):
        tsl = slice(tt * NP, (tt + 1) * NP)
        psv = projps.tile([NP, CD], F32, tag="proj")
        psb = bqkps.tile([NP, HL], F32, tag="bqkps")
        for d in range(NDC):
            nc.tensor.matmul(
                psv[:], xt_sb[:, d, tsl], wv_sb[:, d, :],
                start=(d == 0), stop=(d == NDC - 1),
            )
            nc.tensor.matmul(
                psb[:], xt_sb[:, d, tsl], wkb_sb[:, d, :],
                start=(d == 0), stop=(d == NDC - 1),
            )
        vt = v_sb[tt]
        nc.vector.tensor_copy(vt[:, :, 0:DK], psv.rearrange("p (h k) -> p h k", h=HL))
        nc.vector.memset(vt[:, :, DK : DK + 1], 1.0)
        nc.vector.tensor_copy(bqk_sb[:, tt, :], psb[:])

    bqkps_cm.__exit__(None, None, None)
    wvp_cm.__exit__(None, None, None)

    attnps_cm = tc.tile_pool(name="attnps", bufs=1, space="PSUM")
    attnps = attnps_cm.__enter__()

    def score_tile(h):
        return attnps.tile([NP, QB], F32, tag=f"s{h}", bufs=2, name=f"s{h}")

    def pv_tile(h):
        return attnps.tile([DK + 1, QB], F32, tag=f"pv{h}", bufs=1, name=f"pv{h}")

    # ---- Q/K projection steps (also used as PE filler during attention) ----
    qt_t = [None] * NPAIR
    kt_t = [None] * NPAIR

    def alloc_qk(c):
        qt_t[c] = qkp.tile([NP, T], BF16, tag="qt", name=f"qt{c}")
        kt_t[c] = qkp.tile([NP, T], BF16, tag="kt", name=f"kt{c}")

    def emit_proj_step(c, which, tb):
        bsl = slice(tb * QB, (tb + 1) * QB)
        w_sb = wq_t[c] if which == "q" else wk_t[c]
        dst = qt_t[c] if which == "q" else kt_t[c]
        ps = projps.tile([NP, QB], F32, tag="proj")
        for d in range(NDC):
            nc.tensor.matmul(
                ps[:], w_sb[:, d, :], xt_sb[:, d, bsl],
                start=(d == 0), stop=(d == NDC - 1),
            )
        nc.vector.tensor_copy(dst[:, bsl], ps[:])

    def proj_steps(c):
        for tb in range(NTB):
            yield ("q", tb)
            yield ("k", tb)

    # pair 0 projections up front
    alloc_qk(0)
    for which, tb in proj_steps(0):
        emit_proj_step(0, which, tb)

    oat = [
        oatp.tile([NP, T], BF16, tag=f"oat{c}", name=f"oat{c}") for c in range(NPAIR)
    ]

    # ---- attention per pair, with next-pair projections as filler ----
    for c in range(NPAIR):
        if c + 1 < NPAIR:
            emit_wqk_dma(c + 1)
            alloc_qk(c + 1)
            filler = proj_steps(c + 1)
        else:
            filler = iter(())
        qt, kt = qt_t[c], kt_t[c]
        nflr = 0
        kt_total = sum(4 * qb + 4 for qb in range(NTB))  # 40
        kt_seen = 0

        pvs = pvsp.tile([NP, NTB, QB], BF16, tag="pvs", name=f"pvs{c}")
        rec_h = [
            dnp.tile([1, T], BF16, tag=f"rec{h}", name=f"rec{h}") for h in range(2)
        ]
        dcp_h = [
            dnp.tile([1, T], F32, tag=f"dcp{h}", name=f"dcp{h}") for h in range(2)
        ]

        for qb in range(NTB):
            qsl0 = qb * QB
            nkt = 4 * qb + 4
            pv = [pv_tile(h) for h in range(2)]
            prev = None  # (kk, o, sps, et)
            for kti in range(nkt):
                di = kti - 4 * qb
                o = max(di, 0) * NP  # first causally valid column
                sps = [score_tile(h) for h in range(2)]
                for h in range(2):
                    nc.tensor.matmul(
                        sps[h][:, o:QB],
                        kt[64 * h : 64 * h + 64, kti * NP : (kti + 1) * NP],
                        qt[64 * h : 64 * h + 64, qsl0 + o : qsl0 + QB],
                        start=True, stop=True,
                        tile_position=(64 * h, 0),
                    )
                if prev is not None:
                    _emit_exp_pv(nc, prev, qb, c, bqk_sb, tri_sb, pv, nkt, v_sb)
                prev = (kti, o, sps, _alloc_et(etp))
                kt_seen += 1
                # interleave next-pair projection work to keep PE dense
                want = (kt_seen * 8) // kt_total
                while nflr < want:
                    try:
                        which, tb = next(filler)
                    except StopIteration:
                        nflr = 8
                        break
                    emit_proj_step(c + 1, which, tb)
                    nflr += 1
            _emit_exp_pv(nc, prev, qb, c, bqk_sb, tri_sb, pv, nkt, v_sb)

            # extract unnormalized out^T (packed heads); 1/sumexp straight from
            # the PSUM ones-row into per-head [1, T] tiles (fp32r bits)
            for h in range(2):
                nc.vector.tensor_copy(pvs[64 * h : 64 * h + 64, qb, :], pv[h][0:DK, :])
                nc.vector.tensor_copy(
                    dcp_h[h][0:1, qb * QB : (qb + 1) * QB], pv[h][DK : DK + 1, :]
                )

        # drain any remaining filler
        for which, tb in filler:
            emit_proj_step(c + 1, which, tb)

        # batched fast reciprocal per head (approx op needs SBUF base-0 src)
        for h in range(2):
            dscr = dnp.tile([1, T], F32, tag="dscr", name="dscr")
            nc.vector.reciprocal_approx_fast(dscr[:], dcp_h[h][:])
            nc.vector.tensor_copy(rec_h[h][:], dscr[:])

        # broadcast 1/d to both heads' partitions and normalize into oat
        for qb in range(NTB):
            bc = projps.tile([NP, QB], F32, tag="proj")
            for h in range(2):
                nc.tensor.matmul(
                    bc[:], sel64[h][:], rec_h[h][0:1, qb * QB : (qb + 1) * QB],
                    start=(h == 0), stop=(h == 1),
                )
            bcs = smallp.tile([NP, QB], BF16, tag="bcs")
            nc.vector.tensor_copy(bcs[:], bc[:])
            nc.vector.tensor_mul(
                oat[c][:, qb * QB : (qb + 1) * QB], pvs[:, qb, :], bcs[:]
            )

    attnps_cm.__exit__(None, None, None)

    # ---- final projection: y[t, :] = sum_c oat[c].T @ Wo[c] ----
    wop_cm = tc.tile_pool(name="wop", bufs=1)
    wop = wop_cm.__enter__()
    wo_sb = wop.tile([NP, NPAIR, D], BF16, tag="wo")
    for cc in range(NPAIR):
        nc.gpsimd.dma_start(wo_sb[:, cc, :], wo[cc * NP : (cc + 1) * NP, :])
    yps_cm = tc.tile_pool(name="yps", bufs=2, space="PSUM")
    yps = yps_cm.__enter__()
    for tt in range(NTT):
        tsl = slice(tt * NP, (tt + 1) * NP)
        yt = yps.tile([NP, D], F32, tag="y")
        for dh in range(2):
            for cc in range(NPAIR):
                nc.tensor.matmul(
                    yt[:, dh * QB : (dh + 1) * QB],
                    oat[cc][:, tsl],
                    wo_sb[:, cc, dh * QB : (dh + 1) * QB],
                    start=(cc == 0), stop=(cc == NPAIR - 1),
                )
        ystage = smallp.tile([NP, D], BF16, tag="ystage")
        nc.vector.tensor_copy(ystage[:], yt[:])
        nc.gpsimd.dma_start(y[tsl, :], ystage[:])
    yps_cm.__exit__(None, None, None)
    wop_cm.__exit__(None, None, None)


def _alloc_et(etp):
    return [etp.tile([NP, QB], BF16, tag=f"e{h}", name=f"e{h}") for h in range(2)]


def _emit_exp_pv(nc, prev, qb, c, bqk_sb, tri_sb, pv, nkt, v_sb):
    """exp -> (triangle zero) -> PV accumulate for one k-tile."""
    kti, o, sps, et = prev
    diag = kti >= 4 * qb
    for h in range(2):
        hh = 2 * c + h
        nc.scalar.activation(
            et[h][:, o:QB],
            sps[h][:, o:QB],
            Exp,
            bias=bqk_sb[:, kti, hh : hh + 1],
            scale=0.125,
        )
        if diag:
            nc.vector.tensor_mul(
                et[h][:, o : o + NP], et[h][:, o : o + NP], tri_sb[:]
            )
        nc.tensor.matmul(
            pv[h][:, o:QB],
            v_sb[kti][:, hh, 0 : DK + 1],
            et[h][:, o:QB],
            start=(kti == 0), stop=(kti == nkt - 1),
        )


def _install_ntff_hook_shim():
    """Provide the missing axon_hooks module so trace=True works under axon."""
    try:
        import sys
        import types

        if "antenv.axon_hooks" not in sys.modules:
            mod = types.ModuleType("antenv.axon_hooks")
            mod._hook = None
            mod.set_axon_ntff_profile_hook = lambda h: setattr(mod, "_hook", h)
            mod.get_axon_ntff_profile_hook = lambda: mod._hook
            sys.modules["antenv.axon_hooks"] = mod
            import antenv

            antenv.axon_hooks = mod
        from antenv.axon_hooks import (
            get_axon_ntff_profile_hook,
            set_axon_ntff_profile_hook,
        )

        if get_axon_ntff_profile_hook() is None:
            from trn_agent_boot.trn_boot import _ntff_profile_via_ctypes

            hook = _ntff_profile_via_ctypes("/opt/axon/libaxon_pjrt.so")
            if hook is not None:
                set_axon_ntff_profile_hook(hook)
    except Exception as e:  # noqa: BLE001
        print(f"ntff hook shim failed ({e}); running without trace")


def _round_f32r(a: np.ndarray) -> np.ndarray:
    """Round fp32 to fp32r (11-bit mantissa, low 12 bits zero), RNE."""
    u = np.ascontiguousarray(a, dtype=np.float32).view(np.uint32)
    u = (u + np.uint32(0x7FF) + ((u >> np.uint32(12)) & np.uint32(1))) & np.uint32(
        0xFFFFF000
    )
    return u.view(np.float32)


def _bf(a: np.ndarray) -> np.ndarray:
    return np.ascontiguousarray(a, dtype=np.float32).astype(ml_dtypes.bfloat16)


def kernel(x, Wq, bq, Wk, bk, Wv, bv, Wo, bo):
    x = np.ascontiguousarray(np.asarray(x, dtype=np.float32))
    Wq, bq = np.asarray(Wq, np.float32), np.asarray(bq, np.float32)
    Wk, bk = np.asarray(Wk, np.float32), np.asarray(bk, np.float32)
    Wv, bv = np.asarray(Wv, np.float32), np.asarray(bv, np.float32)
    Wo, bo = np.asarray(Wo, np.float32), np.asarray(bo, np.float32)

    if "nc" not in _CACHE:
        _CACHE["nc"] = _build_nc()
    nc = _CACHE["nc"]

    kk = np.arange(NP)[:, None]
    qq = np.arange(NP)[None, :]
    tri_np = (qq >= kk).astype(np.float32)

    in_maps = []
    for core in range(8):
        b, hg = core // 2, core % 2
        cs = slice(hg * CD, (hg + 1) * CD)
        Wk_c = Wk[:, cs]
        bq_c = bq[cs]
        # wkb[:, h] = 0.125 * Wk_h @ bq_h  (per local head)
        wkb_np = np.stack(
            [
                0.125 * (Wk_c[:, h * DK : (h + 1) * DK] @ bq_c[h * DK : (h + 1) * DK])
                for h in range(HL)
            ],
            axis=1,
        )
        in_maps.append(
            {
                "xt": _bf(x[b].T),
                "wq": _bf(Wq[:, cs]),
                "wk": _bf(Wk_c),
                "wv": _bf(Wv[:, cs]),
                "wkb": _bf(wkb_np),
                "wo": _bf(Wo[cs, :]),
                "tri": _bf(tri_np),
            }
        )

    trace = bool(os.environ.get("KERNEL_TRACE"))
    if trace:
        _install_ntff_hook_shim()
    res = run_bass_kernel_spmd(nc, in_maps, core_ids=list(range(8)), trace=trace)
    _CACHE["last_results"] = res

    bo_eff = bo + bv @ Wo
    out = np.empty((B, T, D), dtype=np.float32)
    for b in range(B):
        out[b] = (
            res.results[2 * b]["y"].astype(np.float32)
            + res.results[2 * b + 1]["y"].astype(np.float32)
            + bo_eff
        )
    return out
